# revision 4
# baseline (speedup 1.0000x reference)
"""HTNE loss kernel for Trainium2, 8-core data-parallel.

Strategy:
  - Shard batch B=131072 across 8 cores (16384 each); replicate emb/delta tables.
  - Per core: batch elements laid out [128 partitions, 128 columns]
    (partition-major: partition p owns elements p*128 .. p*128+127).
  - Embedding rows fetched with gpsimd indirect DMA (128 rows / call).
  - Compute: DVE subtract, ACT square, DVE segmented reduce, softmax-over-H
    and Hawkes weighting via small fused ops, ACT softplus epilogue.
"""

import numpy as np

import concourse.bass as bass
import concourse.mybir as mybir
import concourse.tile as tile
from concourse import bacc
from concourse.bass_utils import run_bass_kernel_spmd

B, H, D, N = 131072, 5, 128, 500000
NCORES = 8
BC = B // NCORES      # 16384 per core
P = 128               # SBUF partitions
C = BC // P           # 128 batch columns per partition
G = 8                 # columns per supertile
NST = C // G          # supertiles per core

f32 = mybir.dt.float32
i32 = mybir.dt.int32
Alu = mybir.AluOpType
Act = mybir.ActivationFunctionType


def build_kernel(tc, nc, io):
    ctxpools = {}
    s_idx, t_idx, h_idx = io["s_idx"], io["t_idx"], io["h_idx"]
    edge_t, h_t, h_m = io["edge_t"], io["h_t"], io["h_m"]
    sgn, emb, delta_tab, loss = io["sgn"], io["emb"], io["delta_tab"], io["loss"]

    with (
        tc.tile_pool(name="idx", bufs=3) as idxp,
        tc.tile_pool(name="gath", bufs=3) as gathp,
        tc.tile_pool(name="small", bufs=4) as smallp,
        tc.tile_pool(name="const", bufs=1) as constp,
    ):
        sgn_t = constp.tile([P, 1], f32)
        nc.sync.dma_start(out=sgn_t[:], in_=sgn[:])

        for st in range(NST):
            cs = st * G
            # ---- dense per-element loads -------------------------------
            sidx = idxp.tile([P, G], i32, tag="sidx")
            tidx = idxp.tile([P, G], i32, tag="tidx")
            hidx = idxp.tile([P, G, H], i32, tag="hidx")
            edge = idxp.tile([P, G], f32, tag="edge")
            ht = idxp.tile([P, G, H], f32, tag="ht")
            hm = idxp.tile([P, G, H], f32, tag="hm")
            nc.sync.dma_start(out=sidx[:], in_=s_idx[:, cs : cs + G])
            nc.sync.dma_start(out=tidx[:], in_=t_idx[:, cs : cs + G])
            nc.sync.dma_start(out=hidx[:], in_=h_idx[:, cs : cs + G, :])
            nc.sync.dma_start(out=edge[:], in_=edge_t[:, cs : cs + G])
            nc.sync.dma_start(out=ht[:], in_=h_t[:, cs : cs + G, :])
            nc.sync.dma_start(out=hm[:], in_=h_m[:, cs : cs + G, :])

            # ---- gathers ----------------------------------------------
            semb = gathp.tile([P, G, D], f32, tag="semb")
            temb = gathp.tile([P, G, D], f32, tag="temb")
            hemb = gathp.tile([P, G, H, D], f32, tag="hemb")
            dlt = idxp.tile([P, G], f32, tag="dlt")
            for g in range(G):
                nc.gpsimd.indirect_dma_start(
                    out=semb[:, g, :],
                    out_offset=None,
                    in_=emb[:],
                    in_offset=bass.IndirectOffsetOnAxis(ap=sidx[:, g : g + 1], axis=0),
                )
                nc.gpsimd.indirect_dma_start(
                    out=temb[:, g, :],
                    out_offset=None,
                    in_=emb[:],
                    in_offset=bass.IndirectOffsetOnAxis(ap=tidx[:, g : g + 1], axis=0),
                )
                nc.gpsimd.indirect_dma_start(
                    out=dlt[:, g : g + 1],
                    out_offset=None,
                    in_=delta_tab[:],
                    in_offset=bass.IndirectOffsetOnAxis(ap=sidx[:, g : g + 1], axis=0),
                )
                for j in range(H):
                    nc.gpsimd.indirect_dma_start(
                        out=hemb[:, g, j, :],
                        out_offset=None,
                        in_=emb[:],
                        in_offset=bass.IndirectOffsetOnAxis(
                            ap=hidx[:, g, j : j + 1], axis=0
                        ),
                    )

            # ---- squared distances ------------------------------------
            # p_mu = -sum((s-t)^2) ; alpha_j = -sum((s-h_j)^2)
            # diff and square computed in place in the gathered tiles:
            # hemb <- (s - h)^2, temb <- (s - t)^2.
            nc.vector.tensor_tensor(
                out=hemb[:],
                in0=semb[:, :, None, :].broadcast_to((P, G, H, D)),
                in1=hemb[:],
                op=Alu.subtract,
            )
            nc.scalar.activation(hemb[:], hemb[:], Act.Square)
            apos = smallp.tile([P, G, H], f32, tag="apos")  # -alpha_j
            nc.vector.tensor_reduce(
                out=apos[:], in_=hemb[:], axis=mybir.AxisListType.X, op=Alu.add
            )

            nc.vector.tensor_tensor(
                out=temb[:], in0=semb[:], in1=temb[:], op=Alu.subtract
            )
            nc.scalar.activation(temb[:], temb[:], Act.Square)
            pmu_pos = smallp.tile([P, G], f32, tag="pmu_pos")  # sum (s-t)^2 >= 0
            nc.vector.tensor_reduce(
                out=pmu_pos[:], in_=temb[:], axis=mybir.AxisListType.X, op=Alu.add
            )

            # ---- softmax over H (alpha = -apos) -----------------------
            # max(alpha) = -min(apos); e = exp(min(apos) - apos)
            mn = smallp.tile([P, G], f32, tag="mn")
            nc.vector.tensor_reduce(
                out=mn[:], in_=apos[:], axis=mybir.AxisListType.X, op=Alu.min
            )
            emx = smallp.tile([P, G, H], f32, tag="emx")
            nc.vector.tensor_tensor(
                out=emx[:],
                in0=mn[:, :, None].broadcast_to((P, G, H)),
                in1=apos[:],
                op=Alu.subtract,
            )
            e = smallp.tile([P, G, H], f32, tag="e")
            nc.scalar.activation(e[:], emx[:], Act.Exp)
            se = smallp.tile([P, G], f32, tag="se")
            nc.vector.tensor_reduce(
                out=se[:], in_=e[:], axis=mybir.AxisListType.X, op=Alu.add
            )
            rcp = smallp.tile([P, G], f32, tag="rcp")
            nc.vector.reciprocal(out=rcp[:], in_=se[:])

            # ---- Hawkes decay: exp(-delta * (edge - h_t)) -------------
            d1 = smallp.tile([P, G, H], f32, tag="d1")
            nc.vector.tensor_tensor(
                out=d1[:],
                in0=ht[:],
                in1=edge[:, :, None].broadcast_to((P, G, H)),
                op=Alu.subtract,
            )
            arg = smallp.tile([P, G, H], f32, tag="arg")
            nc.vector.tensor_tensor(
                out=arg[:],
                in0=d1[:],
                in1=dlt[:, :, None].broadcast_to((P, G, H)),
                op=Alu.mult,
            )
            et = smallp.tile([P, G, H], f32, tag="et")
            nc.scalar.activation(et[:], arg[:], Act.Exp)

            # ---- weighted sum -----------------------------------------
            # sum_j attn_j * alpha_j * et_j * m_j
            #   = -(1/se) * sum_j e_j * apos_j * et_j * m_j
            w = smallp.tile([P, G, H], f32, tag="w")
            nc.vector.tensor_tensor(out=w[:], in0=e[:], in1=apos[:], op=Alu.mult)
            nc.vector.tensor_tensor(out=w[:], in0=w[:], in1=et[:], op=Alu.mult)
            nc.vector.tensor_tensor(out=w[:], in0=w[:], in1=hm[:], op=Alu.mult)
            ws = smallp.tile([P, G], f32, tag="ws")
            nc.vector.tensor_reduce(
                out=ws[:], in_=w[:], axis=mybir.AxisListType.X, op=Alu.add
            )
            # q = -p_lambda = pmu_pos + ws * rcp
            q = smallp.tile([P, G], f32, tag="q")
            nc.vector.tensor_tensor(out=q[:], in0=ws[:], in1=rcp[:], op=Alu.mult)
            nc.vector.tensor_tensor(out=q[:], in0=q[:], in1=pmu_pos[:], op=Alu.add)
            # loss = softplus(sign * q)
            z = smallp.tile([P, G], f32, tag="z")
            nc.vector.tensor_scalar(
                out=z[:], in0=q[:], scalar1=sgn_t[:], scalar2=None, op0=Alu.mult
            )
            # softplus(z) = ln(1 + exp(z)); z is bounded (|z| <~ 15) so the
            # naive form is safe in f32. exp/ln/square live in one ACT table.
            ez = smallp.tile([P, G], f32, tag="ez")
            nc.scalar.activation(ez[:], z[:], Act.Exp)
            nc.vector.tensor_scalar_add(out=ez[:], in0=ez[:], scalar1=1.0)
            lt = smallp.tile([P, G], f32, tag="lt")
            nc.scalar.activation(lt[:], ez[:], Act.Ln)
            nc.sync.dma_start(out=loss[:, cs : cs + G], in_=lt[:])


_PROGRAM_CACHE = {}


def build_program():
    if "nc" in _PROGRAM_CACHE:
        return _PROGRAM_CACHE["nc"]
    nc = bacc.Bacc(
        "TRN2", target_bir_lowering=False, debug=False, num_devices=NCORES
    )
    io = {
        "s_idx": nc.dram_tensor("s_idx", [P, C], i32, kind="ExternalInput").ap(),
        "t_idx": nc.dram_tensor("t_idx", [P, C], i32, kind="ExternalInput").ap(),
        "h_idx": nc.dram_tensor("h_idx", [P, C, H], i32, kind="ExternalInput").ap(),
        "edge_t": nc.dram_tensor("edge_t", [P, C], f32, kind="ExternalInput").ap(),
        "h_t": nc.dram_tensor("h_t", [P, C, H], f32, kind="ExternalInput").ap(),
        "h_m": nc.dram_tensor("h_m", [P, C, H], f32, kind="ExternalInput").ap(),
        "sgn": nc.dram_tensor("sgn", [P, 1], f32, kind="ExternalInput").ap(),
        "emb": nc.dram_tensor("emb", [N, D], f32, kind="ExternalInput").ap(),
        "delta_tab": nc.dram_tensor("delta_tab", [N, 1], f32, kind="ExternalInput").ap(),
        "loss": nc.dram_tensor("loss", [P, C], f32, kind="ExternalOutput").ap(),
    }
    with tile.TileContext(nc) as tc:
        build_kernel(tc, nc, io)
    nc.compile()
    _PROGRAM_CACHE["nc"] = nc
    return nc


def _shard_inputs(s, t, edge_times, h_s, h_s_times, h_s_mask, sign, emb, delta_tab):
    s = np.ascontiguousarray(np.asarray(s, dtype=np.int32))
    t = np.ascontiguousarray(np.asarray(t, dtype=np.int32))
    h_s = np.ascontiguousarray(np.asarray(h_s, dtype=np.int32))
    edge_times = np.ascontiguousarray(np.asarray(edge_times, dtype=np.float32))
    h_s_times = np.ascontiguousarray(np.asarray(h_s_times, dtype=np.float32))
    h_s_mask = np.ascontiguousarray(np.asarray(h_s_mask, dtype=np.float32))
    sign_v = float(np.asarray(sign).reshape(-1)[0])
    emb = np.ascontiguousarray(np.asarray(emb, dtype=np.float32))
    delta_tab = np.ascontiguousarray(np.asarray(delta_tab, dtype=np.float32))

    sgn_full = np.full((P, 1), sign_v, dtype=np.float32)
    in_maps = []
    for c in range(NCORES):
        sl = slice(c * BC, (c + 1) * BC)
        in_maps.append(
            {
                "s_idx": s[sl].reshape(P, C),
                "t_idx": t[sl].reshape(P, C),
                "h_idx": h_s[sl].reshape(P, C, H),
                "edge_t": edge_times[sl].reshape(P, C),
                "h_t": h_s_times[sl].reshape(P, C, H),
                "h_m": h_s_mask[sl].reshape(P, C, H),
                "sgn": sgn_full,
                "emb": emb,
                "delta_tab": delta_tab,
            }
        )
    return in_maps


def run(trace=False, **inputs):
    nc = build_program()
    in_maps = _shard_inputs(**inputs)
    res = run_bass_kernel_spmd(
        nc, in_maps, core_ids=list(range(NCORES)), trace=trace
    )
    out = np.concatenate(
        [res.results[c]["loss"].reshape(BC) for c in range(NCORES)]
    ).astype(np.float32)
    return out, res


def kernel(**inputs):
    out, _ = run(trace=False, **inputs)
    return out
